# revision 1
# baseline (speedup 1.0000x reference)
"""Mixtral sparse-MoE block on 8 TRN2 NeuronCores (expert-parallel).

Strategy: core e owns expert e. Every core computes the (tiny, replicated)
router and its expert's dense SwiGLU FFN over all T tokens, scales rows by
its combine weight column (zero for unselected tokens), and the 8 partial
outputs are AllReduce-summed on-device.

Layouts (host-side, data-independent prep only):
  xT   [H, T]  fp32   hidden_states transposed (replicated)
  gwT  [H, E]  fp32   gate_w transposed (replicated)
  esel [128,E] fp32   one-hot row for this core's expert id
  w1T  [H, F]  fp32   w1[e].T    (matmul-1 stationary, consumed as float32r)
  w3T  [H, F]  fp32   w3[e].T
  w2T  [F, H]  fp16   w2[e].T    (matmul-2 moving operand)

Math per core:
  aT[F,T] = w1T.T.T… : PSUM aT tile = lhsT(w1T[Hk,Fi]).T @ rhs(xT[Hk,Tblk])
  hT = silu(aT) * bT                  (ACT Silu + DVE mul, fp16 out)
  y[T,H] += lhsT(hT[Fk,Ti]).T @ rhs(w2T[Fk,Hc])   (PSUM accum over F)
  y *= combine[:, e]  (per-partition scale on ACT during PSUM evacuation)
  AllReduce(add) over the 8 cores' y partials.
"""

import numpy as np

import concourse.bacc as bacc
import concourse.mybir as mybir
import concourse.tile as tile
from concourse.bass_utils import run_bass_kernel_spmd

F32 = mybir.dt.float32
F32R = mybir.dt.float32r
F16 = mybir.dt.float16

T, H, E = 4096, 2048, 8
FF = 8192
NCORES = 8

TBLK = 512                 # tokens per block
NTB = T // TBLK            # 8 token blocks
HK = H // 128              # 16 contraction tiles (layer 1 / router)
FK = FF // 128             # 64 contraction tiles (layer 2) / F row tiles
WCH = 256                  # layer-1 weight chunk (F columns per DMA)
NFC = FF // WCH            # 32 chunks
NTS = TBLK // 128          # 4 token sub-tiles per block
NHC = H // 512             # 4 output column chunks


def build_kernel():
    nc = bacc.Bacc(trn_type="TRN2", target_bir_lowering=False, debug=False,
                   num_devices=NCORES)
    xT = nc.dram_tensor("xT", [H, T], F32, kind="ExternalInput").ap()
    gwT = nc.dram_tensor("gwT", [H, E], F32, kind="ExternalInput").ap()
    esel = nc.dram_tensor("esel", [128, E], F32, kind="ExternalInput").ap()
    w1T = nc.dram_tensor("w1T", [H, FF], F32, kind="ExternalInput").ap()
    w3T = nc.dram_tensor("w3T", [H, FF], F32, kind="ExternalInput").ap()
    w2T = nc.dram_tensor("w2T", [FF, H], F16, kind="ExternalInput").ap()
    out = nc.dram_tensor("out", [T, H], F32, kind="ExternalOutput").ap()

    xTr = xT.bitcast(F32R)
    w1Tr = w1T.bitcast(F32R)
    w3Tr = w3T.bitcast(F32R)
    gwTr = gwT.bitcast(F32R)

    with tile.TileContext(nc) as tc:
        with (
            tc.tile_pool(name="const", bufs=1) as constp,
            tc.tile_pool(name="xt", bufs=1) as xtp,
            tc.tile_pool(name="w13", bufs=2) as w13p,
            tc.tile_pool(name="ht", bufs=1) as htp,
            tc.tile_pool(name="w2", bufs=4) as w2p,
            tc.tile_pool(name="yrow", bufs=2) as yrowp,
            tc.tile_pool(name="silu", bufs=3) as silup,
            tc.tile_pool(name="rt", bufs=2) as rtp,
            tc.tile_pool(name="psAB", bufs=2, space="PSUM") as psab,
            tc.tile_pool(name="psY", bufs=2, space="PSUM") as psy,
            tc.tile_pool(name="psL", bufs=2, space="PSUM") as psl,
            tc.tile_pool(name="dram", bufs=1, space="DRAM") as dramp,
        ):
            part = dramp.tile([T, H], F32)

            # --- replicated constants ---
            gw_t = []
            for hk in range(HK):
                g = constp.tile([128, E], F32R, tag=f"gw{hk}")
                nc.sync.dma_start(out=g[:], in_=gwTr[hk * 128:(hk + 1) * 128, :])
                gw_t.append(g)
            esel_t = constp.tile([128, E], F32, tag="esel")
            nc.sync.dma_start(out=esel_t[:], in_=esel)

            for tb in range(NTB):
                t0 = tb * TBLK

                # --- activations for this block: xT[:, t0:t0+TBLK] ---
                xt = []
                for hk in range(HK):
                    xx = xtp.tile([128, TBLK], F32R, tag=f"xt{hk}")
                    nc.sync.dma_start(
                        out=xx[:],
                        in_=xTr[hk * 128:(hk + 1) * 128, t0:t0 + TBLK])
                    xt.append(xx)

                # --- router: combine weight column for this expert ---
                ccols = []
                for ts_ in range(NTS):
                    lg = psl.tile([128, E], F32, tag="lg")
                    for hk in range(HK):
                        nc.tensor.matmul(
                            lg[:], xt[hk][:, ts_ * 128:(ts_ + 1) * 128], gw_t[hk][:],
                            start=(hk == 0), stop=(hk == HK - 1))
                    nm = rtp.tile([128, 1], F32, tag="nm")
                    nc.vector.tensor_reduce(nm[:], lg[:], axis=mybir.AxisListType.X,
                                            op=mybir.AluOpType.max, negate=True)
                    ex = rtp.tile([128, E], F32, tag="ex")
                    nc.scalar.activation(ex[:], lg[:],
                                         mybir.ActivationFunctionType.Exp,
                                         bias=nm[:], scale=1.0)
                    m1 = rtp.tile([128, 1], F32, tag="m1")
                    nc.vector.tensor_reduce(m1[:], ex[:], axis=mybir.AxisListType.X,
                                            op=mybir.AluOpType.max)
                    mlt = rtp.tile([128, E], F32, tag="mlt")
                    nc.vector.tensor_scalar(mlt[:], ex[:], m1[:], None,
                                            op0=mybir.AluOpType.is_lt)
                    e2 = rtp.tile([128, E], F32, tag="e2")
                    nc.vector.tensor_tensor(e2[:], ex[:], mlt[:],
                                            op=mybir.AluOpType.mult)
                    m2 = rtp.tile([128, 1], F32, tag="m2")
                    nc.vector.tensor_reduce(m2[:], e2[:], axis=mybir.AxisListType.X,
                                            op=mybir.AluOpType.max)
                    d = rtp.tile([128, 1], F32, tag="d")
                    nc.vector.tensor_tensor(d[:], m1[:], m2[:],
                                            op=mybir.AluOpType.add)
                    r = rtp.tile([128, 1], F32, tag="r")
                    nc.vector.reciprocal(r[:], d[:])
                    mge = rtp.tile([128, E], F32, tag="mge")
                    nc.vector.tensor_scalar(mge[:], ex[:], m2[:], None,
                                            op0=mybir.AluOpType.is_ge)
                    cw = rtp.tile([128, E], F32, tag="cw")
                    nc.vector.tensor_tensor(cw[:], ex[:], mge[:],
                                            op=mybir.AluOpType.mult)
                    # pick this expert's column, scale by 1/(m1+m2)
                    cs = rtp.tile([128, E], F32, tag="cs")
                    nc.vector.tensor_tensor(cs[:], cw[:], esel_t[:],
                                            op=mybir.AluOpType.mult)
                    csum = rtp.tile([128, 1], F32, tag="csum")
                    nc.vector.tensor_reduce(csum[:], cs[:], axis=mybir.AxisListType.X,
                                            op=mybir.AluOpType.add)
                    cc = rtp.tile([128, 1], F32, tag=f"cc{ts_}")
                    nc.vector.tensor_tensor(cc[:], csum[:], r[:],
                                            op=mybir.AluOpType.mult)
                    ccols.append(cc)

                # --- layer 1: hT[f, t] = silu(w1T.T x) * (w3T.T x), fp16 ---
                ht = []
                for fc in range(NFC):
                    w1c, w3c = [], []
                    for hk in range(HK):
                        w1t_ = w13p.tile([128, WCH], F32R, tag=f"w1c{hk}")
                        nc.sync.dma_start(
                            out=w1t_[:],
                            in_=w1Tr[hk * 128:(hk + 1) * 128,
                                     fc * WCH:(fc + 1) * WCH])
                        w1c.append(w1t_)
                        w3t_ = w13p.tile([128, WCH], F32R, tag=f"w3c{hk}")
                        nc.sync.dma_start(
                            out=w3t_[:],
                            in_=w3Tr[hk * 128:(hk + 1) * 128,
                                     fc * WCH:(fc + 1) * WCH])
                        w3c.append(w3t_)
                    for fj in range(WCH // 128):
                        fk = fc * (WCH // 128) + fj
                        psA = psab.tile([128, TBLK], F32, tag="psA")
                        psB = psab.tile([128, TBLK], F32, tag="psB")
                        for hk in range(HK):
                            nc.tensor.matmul(
                                psA[:], w1c[hk][:, fj * 128:(fj + 1) * 128],
                                xt[hk][:], start=(hk == 0), stop=(hk == HK - 1))
                        for hk in range(HK):
                            nc.tensor.matmul(
                                psB[:], w3c[hk][:, fj * 128:(fj + 1) * 128],
                                xt[hk][:], start=(hk == 0), stop=(hk == HK - 1))
                        st = silup.tile([128, TBLK], F32, tag="st")
                        nc.scalar.activation(st[:], psA[:],
                                             mybir.ActivationFunctionType.Silu)
                        hh = htp.tile([128, TBLK], F16, tag=f"ht{fk}")
                        nc.vector.tensor_tensor(hh[:], st[:], psB[:],
                                                op=mybir.AluOpType.mult)
                        ht.append(hh)

                # --- layer 2: y[t, h] = hT.T @ w2T, scaled by combine col ---
                for ts_ in range(NTS):
                    yrow = yrowp.tile([128, H], F32, tag="yrow")
                    for hc in range(NHC):
                        ps2 = psy.tile([128, 512], F32, tag="ps2")
                        for fk in range(FK):
                            w2t_ = w2p.tile([128, 512], F16, tag="w2t")
                            nc.sync.dma_start(
                                out=w2t_[:],
                                in_=w2T[fk * 128:(fk + 1) * 128,
                                        hc * 512:(hc + 1) * 512])
                            nc.tensor.matmul(
                                ps2[:], ht[fk][:, ts_ * 128:(ts_ + 1) * 128],
                                w2t_[:], start=(fk == 0), stop=(fk == FK - 1))
                        nc.scalar.activation(
                            yrow[:, hc * 512:(hc + 1) * 512], ps2[:],
                            mybir.ActivationFunctionType.Copy,
                            scale=ccols[ts_][:])
                    r0 = t0 + ts_ * 128
                    nc.sync.dma_start(out=part[r0:r0 + 128, :], in_=yrow[:])

                # --- reduce this block across cores, write to output ---
                blk = part[t0:t0 + TBLK, :]
                nc.gpsimd.collective_compute(
                    "AllReduce", mybir.AluOpType.add,
                    replica_groups=[list(range(NCORES))],
                    ins=[blk.opt()], outs=[blk.opt()])
                nc.sync.dma_start(out=out[t0:t0 + TBLK, :], in_=blk)

    nc.compile()
    return nc


_NC_CACHE = {}


def _get_nc():
    if "nc" not in _NC_CACHE:
        _NC_CACHE["nc"] = build_kernel()
    return _NC_CACHE["nc"]


def kernel(hidden_states, gate_w, w1, w2, w3):
    hidden_states = np.asarray(hidden_states, dtype=np.float32)
    gate_w = np.asarray(gate_w, dtype=np.float32)
    w1 = np.asarray(w1, dtype=np.float32)
    w2 = np.asarray(w2, dtype=np.float32)
    w3 = np.asarray(w3, dtype=np.float32)

    xT = np.ascontiguousarray(hidden_states.T)
    gwT = np.ascontiguousarray(gate_w.T)
    in_maps = []
    for e in range(NCORES):
        esel = np.zeros((128, E), dtype=np.float32)
        esel[:, e] = 1.0
        in_maps.append({
            "xT": xT,
            "gwT": gwT,
            "esel": esel,
            "w1T": np.ascontiguousarray(w1[e].T),
            "w3T": np.ascontiguousarray(w3[e].T),
            "w2T": np.ascontiguousarray(w2[e].T).astype(np.float16),
        })

    nc = _get_nc()
    res = run_bass_kernel_spmd(nc, in_maps, core_ids=list(range(NCORES)))
    return res.results[0]["out"]


# revision 12
# speedup vs baseline: 1.9933x; 1.9933x over previous
"""Mixtral sparse-MoE block on 8 TRN2 NeuronCores (expert-parallel).

Strategy: core e owns expert e. Every core computes the (tiny, replicated)
router in exact fp32 and its expert's dense SwiGLU FFN over all T tokens in
fp16 (fp32 PSUM accumulation), scales rows by its combine-weight column
(zero for unselected tokens), and the 8 partial outputs are AllReduce-summed
on-device per token block.

Host-side prep is layout/dtype only (transposes + fp16 casts), no
data-dependent compute.

Device inputs per core:
  xT    [H, T]  fp32   x transposed (router, exact fp32 logits)
  x16   [H, T]  fp16   x transposed (layer-1 moving operand)
  gwT   [H, E]  fp32   gate transposed
  esel  [128,E] fp32   one-hot row of this core's expert
  w1T   [H, F]  fp16   w1[e].T   (layer-1 stationary)
  w3T   [H, F]  fp16   w3[e].T
  w2T   [F, H]  fp16   w2[e].T   (layer-2 moving operand)
"""

import numpy as np

import concourse.bacc as bacc
import concourse.mybir as mybir
import concourse.tile as tile
from concourse.bass_utils import run_bass_kernel_spmd

F32 = mybir.dt.float32
F16 = mybir.dt.float16

T, H, E = 4096, 2048, 8
FF = 8192
NCORES = 8

TBLK = 512                 # tokens per block
NTB = T // TBLK            # 8 token blocks
HK = H // 128              # 16 contraction tiles (layer 1 / router)
FK = FF // 128             # 64 F row tiles
NTS = TBLK // 128          # 4 token sub-tiles per block
FGRP = 8                   # layer-2 f-group size (fk tiles per group)
NGRP = FK // FGRP          # 8 groups
NHC = H // 512             # 4 output column chunks
HK4 = 4                    # hk tiles per w13 DMA


def build_kernel():
    nc = bacc.Bacc(trn_type="TRN2", target_bir_lowering=False, debug=False,
                   num_devices=NCORES)
    xT = nc.dram_tensor("xT", [H, T], F32, kind="ExternalInput").ap()
    x16 = nc.dram_tensor("x16", [H, T], F16, kind="ExternalInput").ap()
    gwT = nc.dram_tensor("gwT", [H, E], F32, kind="ExternalInput").ap()
    esel = nc.dram_tensor("esel", [128, E], F32, kind="ExternalInput").ap()
    w1T = nc.dram_tensor("w1T", [H, FF], F16, kind="ExternalInput").ap()
    w3T = nc.dram_tensor("w3T", [H, FF], F16, kind="ExternalInput").ap()
    w2T = nc.dram_tensor("w2T", [FF, H], F16, kind="ExternalInput").ap()
    out = nc.dram_tensor("out", [T, H], F32, kind="ExternalOutput").ap()

    with tile.TileContext(nc) as tc:
        with (
            tc.tile_pool(name="const", bufs=1) as constp,
            tc.tile_pool(name="xtr", bufs=3) as xtrp,
            tc.tile_pool(name="xt", bufs=1) as xtp,
            tc.tile_pool(name="w13", bufs=2) as w13p,
            tc.tile_pool(name="ht", bufs=2) as htp,
            tc.tile_pool(name="w2", bufs=1) as w2p,
            tc.tile_pool(name="ysb", bufs=1) as ysbp,
            tc.tile_pool(name="yout", bufs=2) as youtp,
            tc.tile_pool(name="silu", bufs=2) as silup,
            tc.tile_pool(name="rt", bufs=2) as rtp,
            tc.tile_pool(name="psAB", bufs=1, space="PSUM") as psab,
            tc.tile_pool(name="psY", bufs=2, space="PSUM") as psy,
            tc.tile_pool(name="psL", bufs=4, space="PSUM") as psl,
            tc.tile_pool(name="dram", bufs=1, space="DRAM") as dramp,
        ):
            part = dramp.tile([T, H], F32)

            # ---------------- replicated constants ----------------
            gw_t = []
            for hk in range(HK):
                g = constp.tile([128, E], F32, tag=f"gw{hk}")
                nc.sync.dma_start(out=g[:], in_=gwT[hk * 128:(hk + 1) * 128, :])
                gw_t.append(g)
            esel_t = constp.tile([128, E], F32, tag="esel")
            nc.sync.dma_start(out=esel_t[:], in_=esel)

            # ---------------- router phase (exact fp32) ----------------
            # combine column for this core's expert, all T tokens
            ccols = []
            for tq in range(T // TBLK):
                # hk-outer: each xT tile feeds 4 interleaved PSUM accumulation
                # groups (one per token subtile) and is then released.
                lgs = []
                for ts_ in range(NTS):
                    lg = psl.tile([128, E], F32, tag=f"lg{ts_}", name=f"lg{ts_}",
                                  bufs=1)
                    lgs.append(lg)
                for hk in range(HK):
                    xx = xtrp.tile([128, TBLK], F32, tag="xtr")
                    nc.sync.dma_start(
                        out=xx[:],
                        in_=xT[hk * 128:(hk + 1) * 128,
                               tq * TBLK:(tq + 1) * TBLK])
                    for ts_ in range(NTS):
                        nc.tensor.matmul(
                            lgs[ts_][:], xx[:, ts_ * 128:(ts_ + 1) * 128],
                            gw_t[hk][:], start=(hk == 0), stop=(hk == HK - 1))
                for ts_ in range(NTS):
                    tt = tq * NTS + ts_
                    lg = lgs[ts_]
                    nm = rtp.tile([128, 1], F32, tag="nm")
                    nc.vector.tensor_reduce(nm[:], lg[:], axis=mybir.AxisListType.X,
                                            op=mybir.AluOpType.max, negate=True)
                    ex = rtp.tile([128, E], F32, tag="ex")
                    nc.scalar.activation(ex[:], lg[:],
                                         mybir.ActivationFunctionType.Exp,
                                         bias=nm[:], scale=1.0)
                    m1 = rtp.tile([128, 1], F32, tag="m1")
                    nc.vector.tensor_reduce(m1[:], ex[:], axis=mybir.AxisListType.X,
                                            op=mybir.AluOpType.max)
                    mlt = rtp.tile([128, E], F32, tag="mlt")
                    nc.vector.tensor_scalar(mlt[:], ex[:], m1[:], None,
                                            op0=mybir.AluOpType.is_lt)
                    e2 = rtp.tile([128, E], F32, tag="e2")
                    nc.vector.tensor_tensor(e2[:], ex[:], mlt[:],
                                            op=mybir.AluOpType.mult)
                    m2 = rtp.tile([128, 1], F32, tag="m2")
                    nc.vector.tensor_reduce(m2[:], e2[:], axis=mybir.AxisListType.X,
                                            op=mybir.AluOpType.max)
                    d = rtp.tile([128, 1], F32, tag="d")
                    nc.vector.tensor_tensor(d[:], m1[:], m2[:],
                                            op=mybir.AluOpType.add)
                    r = rtp.tile([128, 1], F32, tag="r")
                    nc.vector.reciprocal(r[:], d[:])
                    mge = rtp.tile([128, E], F32, tag="mge")
                    nc.vector.tensor_scalar(mge[:], ex[:], m2[:], None,
                                            op0=mybir.AluOpType.is_ge)
                    cw = rtp.tile([128, E], F32, tag="cw")
                    nc.vector.tensor_tensor(cw[:], ex[:], mge[:],
                                            op=mybir.AluOpType.mult)
                    cs = rtp.tile([128, E], F32, tag="cs")
                    nc.vector.tensor_tensor(cs[:], cw[:], esel_t[:],
                                            op=mybir.AluOpType.mult)
                    csum = rtp.tile([128, 1], F32, tag="csum")
                    nc.vector.tensor_reduce(csum[:], cs[:],
                                            axis=mybir.AxisListType.X,
                                            op=mybir.AluOpType.add)
                    cc = constp.tile([128, 1], F32, tag=f"cc{tt}")
                    nc.vector.tensor_tensor(cc[:], csum[:], r[:],
                                            op=mybir.AluOpType.mult)
                    ccols.append(cc)

            # Scheduler-only fence: keep the router's long serial chains from
            # interleaving with (and resource-deadlocking against) the main loop.
            tc.no_sync_barrier()

            # ---------------- main loop ----------------
            for tb in range(NTB):
                t0 = tb * TBLK

                xt = []
                for hk in range(HK):
                    xx = xtp.tile([128, TBLK], F16, tag=f"xt{hk}")
                    nc.sync.dma_start(
                        out=xx[:],
                        in_=x16[hk * 128:(hk + 1) * 128, t0:t0 + TBLK])
                    xt.append(xx)

                ysb = []
                for ts_ in range(NTS):
                    yt = ysbp.tile([128, H], F32, tag=f"ysb{ts_}", name=f"ysb{ts_}")
                    ysb.append(yt)

                for g in range(NGRP):
                    # ---- layer 1 for this f-group: ht[fk], fk in group ----
                    ht = []
                    for fc in range(FGRP * 128 // 512):   # 512-F chunks: 4
                        f0 = g * FGRP * 128 + fc * 512
                        w1c, w3c = [], []
                        for h4 in range(HK // HK4):       # 4 DMAs of 4 hk
                            wt = w13p.tile([128, HK4, 512], F16, tag=f"w1c{h4}")
                            nc.sync.dma_start(
                                out=wt[:],
                                in_=w1T[h4 * HK4 * 128:(h4 + 1) * HK4 * 128,
                                        f0:f0 + 512].rearrange(
                                            "(k p) f -> p k f", p=128))
                            w1c.append(wt)
                            wt = w13p.tile([128, HK4, 512], F16, tag=f"w3c{h4}")
                            nc.sync.dma_start(
                                out=wt[:],
                                in_=w3T[h4 * HK4 * 128:(h4 + 1) * HK4 * 128,
                                        f0:f0 + 512].rearrange(
                                            "(k p) f -> p k f", p=128))
                            w3c.append(wt)
                        for fj in range(4):               # 128-F subtiles
                            fk = g * FGRP + fc * 4 + fj
                            psA = psab.tile([128, TBLK], F32, tag="psA")
                            psB = psab.tile([128, TBLK], F32, tag="psB")
                            for hk in range(HK):
                                nc.tensor.matmul(
                                    psA[:],
                                    w1c[hk // HK4][:, hk % HK4,
                                                   fj * 128:(fj + 1) * 128],
                                    xt[hk][:],
                                    start=(hk == 0), stop=(hk == HK - 1))
                            for hk in range(HK):
                                nc.tensor.matmul(
                                    psB[:],
                                    w3c[hk // HK4][:, hk % HK4,
                                                   fj * 128:(fj + 1) * 128],
                                    xt[hk][:],
                                    start=(hk == 0), stop=(hk == HK - 1))
                            st = silup.tile([128, TBLK], F32, tag="st")
                            nc.scalar.activation(
                                st[:], psA[:], mybir.ActivationFunctionType.Silu)
                            hh = htp.tile([128, TBLK], F16, tag=f"ht{fk % FGRP}")
                            nc.vector.tensor_tensor(hh[:], st[:], psB[:],
                                                    op=mybir.AluOpType.mult)
                            ht.append(hh)

                    # ---- layer 2 partial: y += ht.T @ w2T over this group ----
                    w2s = []
                    for j in range(FGRP):
                        fk = g * FGRP + j
                        ws = w2p.tile([128, H], F16, tag=f"w2s{j}")
                        nc.gpsimd.dma_start(
                            out=ws[:], in_=w2T[fk * 128:(fk + 1) * 128, :])
                        w2s.append(ws)
                    for ts_ in range(NTS):
                        for hc in range(NHC):
                            ps2 = psy.tile([128, 512], F32, tag="ps2")
                            for j in range(FGRP):
                                nc.tensor.matmul(
                                    ps2[:],
                                    ht[j][:, ts_ * 128:(ts_ + 1) * 128],
                                    w2s[j][:, hc * 512:(hc + 1) * 512],
                                    start=(j == 0), stop=(j == FGRP - 1))
                            dst = ysb[ts_][:, hc * 512:(hc + 1) * 512]
                            if g == 0:
                                nc.vector.tensor_copy(dst, ps2[:])
                            else:
                                nc.vector.tensor_tensor(dst, ps2[:], dst,
                                                        op=mybir.AluOpType.add)

                # ---- scale by combine column, ship out, reduce ----
                for ts_ in range(NTS):
                    yo = youtp.tile([128, H], F32, tag="yout")
                    nc.scalar.mul(yo[:], ysb[ts_][:],
                                  ccols[tb * NTS + ts_][:])
                    r0 = t0 + ts_ * 128
                    nc.sync.dma_start(out=part[r0:r0 + 128, :], in_=yo[:])

                blk = part[t0:t0 + TBLK, :]
                nc.gpsimd.collective_compute(
                    "AllReduce", mybir.AluOpType.add,
                    replica_groups=[list(range(NCORES))],
                    ins=[blk.opt()], outs=[blk.opt()])
                nc.sync.dma_start(out=out[t0:t0 + TBLK, :], in_=blk)

    nc.compile()
    return nc


_NC_CACHE = {}


def _get_nc():
    if "nc" not in _NC_CACHE:
        _NC_CACHE["nc"] = build_kernel()
    return _NC_CACHE["nc"]


def kernel(hidden_states, gate_w, w1, w2, w3):
    hidden_states = np.asarray(hidden_states, dtype=np.float32)
    gate_w = np.asarray(gate_w, dtype=np.float32)
    w1 = np.asarray(w1, dtype=np.float32)
    w2 = np.asarray(w2, dtype=np.float32)
    w3 = np.asarray(w3, dtype=np.float32)

    xT = np.ascontiguousarray(hidden_states.T)
    x16 = xT.astype(np.float16)
    gwT = np.ascontiguousarray(gate_w.T)
    in_maps = []
    for e in range(NCORES):
        esel = np.zeros((128, E), dtype=np.float32)
        esel[:, e] = 1.0
        in_maps.append({
            "xT": xT,
            "x16": x16,
            "gwT": gwT,
            "esel": esel,
            "w1T": np.ascontiguousarray(w1[e].T).astype(np.float16),
            "w3T": np.ascontiguousarray(w3[e].T).astype(np.float16),
            "w2T": np.ascontiguousarray(w2[e].T).astype(np.float16),
        })

    nc = _get_nc()
    res = run_bass_kernel_spmd(nc, in_maps, core_ids=list(range(NCORES)))
    return res.results[0]["out"]
